# revision 1
# baseline (speedup 1.0000x reference)
"""DKVMN forward kernel for Trainium2, 8-core batch-parallel.

Model (per sample): T=200 sequential DKVMN memory steps over state
Mv [M=64, D=64], with read-before-update, plus embedding gathers and
small projections around the recurrence.

Sharding: data-parallel over batch. 64 samples -> 8 cores x 8 samples.
Parameters (embedding tables + small weights) replicated per core.

Row space for the parallel phases is SAMPLE-MAJOR with per-sample pad:
row = b_loc*208 + t  (b_loc = g*2 + s; t in [0,208), pad t>=200).
8*208 = 1664 = 13 blocks of 128. Pad rows compute garbage that is
never read back. Sample-major keeps every (g,s)-slice t-contiguous,
so all layout shuffles are plain <=3-dim DMAs.

Per-core recurrence layout (state S[(s,d), (g,m)] = [128, 256] f32):
  per step t:
    Wbc[128,256] = PE broadcast of w_t (indicator matmul, exact fp32)
    P1 = S * Wbc                      (DVE)
    read4[:,g]   = sum_m P1           (DVE segmented reduce)
    S = S - P1 * e_t[d-bcast]         (DVE x2, step-0 AP broadcast)
    S = S + Wbc * a_t[d-bcast]        (DVE x2)
"""

import numpy as np

import concourse.bass as bass
import concourse.bacc as bacc
import concourse.tile as tile
from concourse import mybir
from concourse.bass_utils import run_bass_kernel_spmd

F32 = mybir.dt.float32
I32 = mybir.dt.int32
AL = mybir.AluOpType
AF = mybir.ActivationFunctionType
AX = mybir.AxisListType

NUM_SKILLS = 1000
D = 64          # dim_s
M = 64          # size_m
B = 64          # global batch
T = 200
TP = 208        # padded per-sample length
NCORES = 8
BL = B // NCORES          # samples per core = 8
RPAD = BL * TP            # 1664
NBLK = RPAD // 128        # 13
TCH = 25                  # scan-loop w-stream chunk (steps)
NCH = T // TCH            # 8


def _build():
    nc = bacc.Bacc(None, target_bir_lowering=False, debug=False)

    # ---- external I/O ----
    d_idxk = nc.dram_tensor("idxk", [RPAD], I32, kind="ExternalInput")
    d_resp = nc.dram_tensor("resp", [RPAD], I32, kind="ExternalInput")
    d_kemb = nc.dram_tensor("kemb", [NUM_SKILLS, D], F32, kind="ExternalInput")
    d_vemb = nc.dram_tensor("vemb", [2 * NUM_SKILLS, D], F32, kind="ExternalInput")
    d_mkt = nc.dram_tensor("mkt", [D, M], F32, kind="ExternalInput")      # Mk^T
    d_eawt = nc.dram_tensor("eawt", [D, 2 * D], F32, kind="ExternalInput")  # [eW^T|aW^T]
    d_fwt = nc.dram_tensor("fwt", [2 * D, D], F32, kind="ExternalInput")  # fW^T
    d_pwb = nc.dram_tensor("pwb", [128, D], F32, kind="ExternalInput")    # pW bcast
    d_ebc = nc.dram_tensor("ebc", [D, 1], F32, kind="ExternalInput")      # eb col
    d_abc = nc.dram_tensor("abc", [D, 1], F32, kind="ExternalInput")      # ab col
    d_fbb = nc.dram_tensor("fbb", [128, D], F32, kind="ExternalInput")    # fb bcast
    d_pbc = nc.dram_tensor("pbc", [128, 1], F32, kind="ExternalInput")    # pb col
    d_ind2 = nc.dram_tensor("ind2", [2, 128], F32, kind="ExternalInput")
    d_ident = nc.dram_tensor("ident", [128, 128], F32, kind="ExternalInput")
    d_s0 = nc.dram_tensor("s0", [128, 4 * M], F32, kind="ExternalInput")  # Mv0 prelaid
    d_out = nc.dram_tensor("out", [BL, T - 1], F32, kind="ExternalOutput")

    # ---- internal DRAM staging (w only) ----
    d_w = nc.dram_tensor("w_stg", [RPAD, M], F32)
    d_p = nc.dram_tensor("p_stg", [RPAD], F32)

    with tile.TileContext(nc) as tc:
        import contextlib
        with contextlib.ExitStack() as ctx:
            singles = ctx.enter_context(tc.tile_pool(name="singles", bufs=1))

            t_idxk = singles.tile([128, NBLK], I32)
            t_idxv = singles.tile([128, NBLK], I32)
            t_resp = singles.tile([128, NBLK], I32)
            t_mkt = singles.tile([D, M], F32)
            t_eawt = singles.tile([D, 2 * D], F32)
            t_fwt1 = singles.tile([D, D], F32)
            t_fwt2 = singles.tile([D, D], F32)
            t_pwb = singles.tile([128, D], F32)
            t_ebc = singles.tile([D, 1], F32)
            t_abc = singles.tile([D, 1], F32)
            t_fbb = singles.tile([128, D], F32)
            t_pbc = singles.tile([128, 1], F32)
            t_ind2 = singles.tile([2, 128], F32)
            t_ident = singles.tile([128, 128], F32)
            t_kT = singles.tile([D, RPAD], F32)      # k^T, sample-major cols
            t_eT = singles.tile([D, RPAD], F32)      # sigmoid(v@eW^T+eb)^T
            t_aT = singles.tile([D, RPAD], F32)      # tanh(v@aW^T+ab)^T
            t_rdT = singles.tile([D, RPAD], F32)     # reads^T for stage C
            t_S = singles.tile([128, 4 * M], F32)    # recurrence state
            t_E4 = singles.tile([128, 4, T], F32)    # e in [(s,d), g, t]
            t_A4 = singles.tile([128, 4, T], F32)
            t_reads = singles.tile([128, 4, T], F32)
            t_psig = singles.tile([128, NBLK], F32)

            nc.sync.dma_start(out=t_idxk[:], in_=d_idxk[:].rearrange("(c p) -> p c", p=128))
            nc.sync.dma_start(out=t_resp[:], in_=d_resp[:].rearrange("(c p) -> p c", p=128))
            nc.sync.dma_start(out=t_mkt[:], in_=d_mkt[:])
            nc.sync.dma_start(out=t_eawt[:], in_=d_eawt[:])
            nc.sync.dma_start(out=t_fwt1[:], in_=d_fwt[0:D, :])
            nc.sync.dma_start(out=t_fwt2[:], in_=d_fwt[D:2 * D, :])
            nc.sync.dma_start(out=t_pwb[:], in_=d_pwb[:])
            nc.sync.dma_start(out=t_ebc[:], in_=d_ebc[:])
            nc.sync.dma_start(out=t_abc[:], in_=d_abc[:])
            nc.sync.dma_start(out=t_fbb[:], in_=d_fbb[:])
            nc.sync.dma_start(out=t_pbc[:], in_=d_pbc[:])
            nc.sync.dma_start(out=t_ind2[:], in_=d_ind2[:])
            nc.sync.dma_start(out=t_ident[:], in_=d_ident[:])
            nc.sync.dma_start(out=t_S[:], in_=d_s0[:])

            # v-table index: x = skills + NUM_SKILLS * responses
            # (responses in {0,1}, so the >-1 mask in the reference is identity)
            nc.vector.tensor_scalar(out=t_idxv[:], in0=t_resp[:], scalar1=NUM_SKILLS,
                                    scalar2=None, op0=AL.mult)
            nc.vector.tensor_tensor(out=t_idxv[:], in0=t_idxv[:], in1=t_idxk[:], op=AL.add)

            # ================= stage A: gathers, w / e^T / a^T =================
            with tc.tile_pool(name="sa_sb", bufs=3) as sa, \
                 tc.tile_pool(name="sa_ps", bufs=2, space="PSUM") as sap:
                for c in range(NBLK):
                    kg = sa.tile([128, D], F32, tag="kg")
                    vg = sa.tile([128, D], F32, tag="vg")
                    nc.gpsimd.indirect_dma_start(
                        out=kg[:], out_offset=None, in_=d_kemb[:],
                        in_offset=bass.IndirectOffsetOnAxis(ap=t_idxk[:, c:c + 1], axis=0))
                    nc.gpsimd.indirect_dma_start(
                        out=vg[:], out_offset=None, in_=d_vemb[:],
                        in_offset=bass.IndirectOffsetOnAxis(ap=t_idxv[:, c:c + 1], axis=0))
                    kTp = sap.tile([D, 128], F32, tag="ktp", space="PSUM")
                    vTp = sap.tile([D, 128], F32, tag="vtp", space="PSUM")
                    nc.tensor.transpose(out=kTp[:], in_=kg[:], identity=t_ident[:])
                    nc.tensor.transpose(out=vTp[:], in_=vg[:], identity=t_ident[:])
                    nc.scalar.copy(out=t_kT[:, c * 128:(c + 1) * 128], in_=kTp[:])
                    vT = sa.tile([D, 128], F32, tag="vt")
                    nc.scalar.copy(out=vT[:], in_=vTp[:])

                    # logits = k @ Mk^T  -> [128 rows, M]
                    lg = sap.tile([128, M], F32, tag="lg", space="PSUM")
                    nc.tensor.matmul(lg[:], lhsT=t_kT[:, c * 128:(c + 1) * 128],
                                     rhs=t_mkt[:], start=True, stop=True)
                    mx = sa.tile([128, 1], F32, tag="mx")
                    nc.vector.tensor_reduce(out=mx[:], in_=lg[:], axis=AX.X,
                                            op=AL.max, negate=True)
                    wexp = sa.tile([128, M], F32, tag="wexp")
                    sme = sa.tile([128, 1], F32, tag="sme")
                    nc.scalar.activation(out=wexp[:], in_=lg[:], func=AF.Exp,
                                         bias=mx[:], scale=1.0, accum_out=sme[:])
                    rin = sa.tile([128, 1], F32, tag="rin")
                    nc.vector.reciprocal(out=rin[:], in_=sme[:])
                    wb = sa.tile([128, M], F32, tag="wb")
                    nc.vector.tensor_scalar(out=wb[:], in0=wexp[:], scalar1=rin[:],
                                            scalar2=None, op0=AL.mult)
                    nc.sync.dma_start(out=d_w[c * 128:(c + 1) * 128, :], in_=wb[:])

                    # e/a transposed: eaT = [eW^T|aW^T]^T @ v^T -> [(e|a), rows]
                    eaT = sap.tile([2 * D, 128], F32, tag="eat", space="PSUM")
                    nc.tensor.matmul(eaT[:], lhsT=t_eawt[:], rhs=vT[:],
                                     start=True, stop=True)
                    nc.scalar.activation(out=t_eT[:, c * 128:(c + 1) * 128],
                                         in_=eaT[0:D, :], func=AF.Sigmoid,
                                         bias=t_ebc[:], scale=1.0)
                    nc.scalar.activation(out=t_aT[:, c * 128:(c + 1) * 128],
                                         in_=eaT[D:2 * D, :], func=AF.Tanh,
                                         bias=t_abc[:], scale=1.0)

            # ============ stage A2: (s,d)-packed e/a tiles ============
            for g in range(4):
                for s in range(2):
                    col = (g * 2 + s) * TP
                    nc.sync.dma_start(out=t_E4[s * D:(s + 1) * D, g, 0:T],
                                      in_=t_eT[:, col:col + T])
                    nc.sync.dma_start(out=t_A4[s * D:(s + 1) * D, g, 0:T],
                                      in_=t_aT[:, col:col + T])

            # negate e for the gate trick: G = 1 + Wbc*(-e)
            nc.vector.tensor_scalar(out=t_E4[:].rearrange("p g t -> p (g t)"),
                                    in0=t_E4[:].rearrange("p g t -> p (g t)"),
                                    scalar1=-1.0, scalar2=None, op0=AL.mult)

            # ================= stage B: the recurrence =================
            import os as _os
            _nch = int(_os.environ.get("BSTEPS", str(T))) // TCH
            with tc.tile_pool(name="sb_w", bufs=2) as sbw, \
                 tc.tile_pool(name="sb_t", bufs=3) as sbt, \
                 tc.tile_pool(name="sb_ps", bufs=4, space="PSUM") as sbp:
                for ch in range(_nch):
                    wch = sbw.tile([2, TCH, 4, M], F32, tag="wch")
                    for g in range(4):
                        nc.sync.dma_start(
                            out=wch[:, :, g, :],
                            in_=d_w[:].rearrange("(b t) m -> b t m", b=BL)[
                                g * 2:g * 2 + 2, ch * TCH:(ch + 1) * TCH, :])
                    for tt in range(TCH):
                        t = ch * TCH + tt
                        wbc = sbp.tile([128, 4 * M], F32, tag="wbc", space="PSUM")
                        nc.tensor.matmul(
                            wbc[:], lhsT=t_ind2[:],
                            rhs=wch[:, tt, :, :].rearrange("s g m -> s (g m)"),
                            start=True, stop=True)
                        wbc_v = wbc[:].rearrange("p (g m) -> p g m", g=4)
                        # ACT: gate G = 1 - Wbc*e  (per-g, scale is [P,1])
                        gt = sbt.tile([128, 4, M], F32, tag="gt")
                        for g in range(4):
                            nc.scalar.activation(
                                out=gt[:, g, :], in_=wbc_v[:, g, :], func=AF.Copy,
                                bias=1.0, scale=t_E4[:, g, t:t + 1])
                        # DVE: read product + segmented reduce, then apply update
                        p1 = sbt.tile([128, 4 * M], F32, tag="p1")
                        nc.vector.tensor_tensor(out=p1[:], in0=t_S[:], in1=wbc[:], op=AL.mult)
                        nc.vector.tensor_reduce(
                            out=t_reads[:, :, t],
                            in_=p1[:].rearrange("p (g m) -> p g m", g=4),
                            axis=AX.X, op=AL.add)
                        nc.vector.tensor_tensor(
                            out=t_S[:], in0=t_S[:],
                            in1=gt[:].rearrange("p g m -> p (g m)"), op=AL.mult)
                        t2 = sbt.tile([128, 4, M], F32, tag="t2")
                        nc.vector.tensor_tensor(
                            out=t2[:],
                            in0=wbc_v[:],
                            in1=t_A4[:, :, t].unsqueeze(2).broadcast_to([128, 4, M]),
                            op=AL.mult)
                        nc.vector.tensor_tensor(
                            out=t_S[:], in0=t_S[:],
                            in1=t2[:].rearrange("p g m -> p (g m)"), op=AL.add)

            # reads -> [D, RPAD] sample-major for stage C
            for g in range(4):
                for s in range(2):
                    col = (g * 2 + s) * TP
                    nc.sync.dma_start(out=t_rdT[:, col:col + T],
                                      in_=t_reads[s * D:(s + 1) * D, g, 0:T])

            # ================= stage C: output head =================
            with tc.tile_pool(name="sc_sb", bufs=3) as sc, \
                 tc.tile_pool(name="sc_ps", bufs=2, space="PSUM") as scp:
                for c in range(NBLK):
                    fp = scp.tile([128, D], F32, tag="fp", space="PSUM")
                    nc.tensor.matmul(fp[:], lhsT=t_rdT[:, c * 128:(c + 1) * 128],
                                     rhs=t_fwt1[:], start=True, stop=False)
                    nc.tensor.matmul(fp[:], lhsT=t_kT[:, c * 128:(c + 1) * 128],
                                     rhs=t_fwt2[:], start=False, stop=True)
                    fb = sc.tile([128, D], F32, tag="fb")
                    nc.vector.tensor_tensor(out=fb[:], in0=fp[:], in1=t_fbb[:], op=AL.add)
                    ft = sc.tile([128, D], F32, tag="ft")
                    nc.scalar.activation(out=ft[:], in_=fb[:], func=AF.Tanh)
                    junk = sc.tile([128, D], F32, tag="junk")
                    nc.vector.scalar_tensor_tensor(
                        out=junk[:], in0=ft[:], scalar=1.0, in1=t_pwb[:],
                        op0=AL.mult, op1=AL.mult,
                        accum_out=t_psig[:, c:c + 1])
                nc.scalar.activation(out=t_psig[:], in_=t_psig[:], func=AF.Sigmoid,
                                     bias=t_pbc[:], scale=1.0)
                nc.sync.dma_start(out=d_p[:].rearrange("(c p) -> p c", p=128), in_=t_psig[:])

                # out[b, j] = p[b*208 + 1 + j]
                ob = sc.tile([BL, T - 1], F32, tag="ob")
                nc.sync.dma_start(
                    out=ob[:],
                    in_=d_p[:].rearrange("(b t) -> b t", b=BL)[:, 1:T])
                nc.sync.dma_start(out=d_out[:], in_=ob[:])

    nc.compile()
    return nc


_NC_CACHE = None


def _get_nc():
    global _NC_CACHE
    if _NC_CACHE is None:
        _NC_CACHE = _build()
    return _NC_CACHE


def kernel(skills, responses, k_emb, v_emb, Mk, Mv0, fW, fb, eW, eb, aW, ab, pW, pb):
    skills = np.asarray(skills)
    responses = np.asarray(responses)
    k_emb = np.asarray(k_emb, dtype=np.float32)
    v_emb = np.asarray(v_emb, dtype=np.float32)
    Mk = np.asarray(Mk, dtype=np.float32)
    Mv0 = np.asarray(Mv0, dtype=np.float32)
    fW = np.asarray(fW, dtype=np.float32)
    fb = np.asarray(fb, dtype=np.float32)
    eW = np.asarray(eW, dtype=np.float32)
    eb = np.asarray(eb, dtype=np.float32)
    aW = np.asarray(aW, dtype=np.float32)
    ab = np.asarray(ab, dtype=np.float32)
    pW = np.asarray(pW, dtype=np.float32)
    pb = np.asarray(pb, dtype=np.float32)

    mkt = np.ascontiguousarray(Mk.T)                                   # [D, M]
    eawt = np.ascontiguousarray(np.concatenate([eW.T, aW.T], axis=1))  # [D, 2D]
    fwt = np.ascontiguousarray(fW.T)                                   # [2D, D]
    pwb = np.broadcast_to(pW, (128, D)).copy()
    ebc = np.ascontiguousarray(eb.reshape(D, 1))
    abc = np.ascontiguousarray(ab.reshape(D, 1))
    fbb = np.broadcast_to(fb[None, :], (128, D)).copy()
    pbc = np.full((128, 1), float(pb[0]), np.float32)
    ind2 = np.zeros((2, 128), np.float32)
    ind2[0, :64] = 1.0
    ind2[1, 64:] = 1.0
    ident = np.eye(128, dtype=np.float32)
    # S0[(s,d),(g,m)] = Mv0[m,d]
    s0 = np.tile(Mv0.T.reshape(1, D, 1, M), (2, 1, 4, 1)).reshape(128, 4 * M)
    s0 = np.ascontiguousarray(s0, dtype=np.float32)

    shared = dict(kemb=k_emb, vemb=v_emb, mkt=mkt, eawt=eawt, fwt=fwt,
                  pwb=pwb, ebc=ebc, abc=abc, fbb=fbb, pbc=pbc, ind2=ind2,
                  ident=ident, s0=s0)

    in_maps = []
    for core in range(NCORES):
        sk = skills[core * BL:(core + 1) * BL].astype(np.int32)
        rs = responses[core * BL:(core + 1) * BL].astype(np.int32)
        idxk = np.zeros((BL, TP), np.int32)
        resp = np.zeros((BL, TP), np.int32)
        idxk[:, :T] = sk          # row = b*208 + t
        resp[:, :T] = rs
        m = dict(shared)
        m["idxk"] = idxk.reshape(-1)
        m["resp"] = resp.reshape(-1)
        in_maps.append(m)

    nc = _get_nc()
    res = run_bass_kernel_spmd(nc, in_maps, core_ids=list(range(NCORES)),
                               **_RUN_KWARGS)
    out = np.concatenate([res.results[i]["out"] for i in range(NCORES)], axis=0)
    global _LAST_RESULT
    _LAST_RESULT = res
    return out.astype(np.float32)


_RUN_KWARGS = {}
_LAST_RESULT = None


def run_traced(**inputs):
    """Run once with NTFF tracing; returns exec_time_ns (or None)."""
    global _RUN_KWARGS
    _RUN_KWARGS = {"trace": True}
    try:
        kernel(**inputs)
    finally:
        _RUN_KWARGS = {}
    return _LAST_RESULT.exec_time_ns if _LAST_RESULT is not None else None



# revision 6
# speedup vs baseline: 1.1002x; 1.1002x over previous
"""DKVMN forward kernel for Trainium2, 8-core batch-parallel, scan-based.

Model (per sample): T=200 sequential DKVMN memory steps over state
Mv [M=64, D=64] with read-before-update, plus embedding gathers and
small projections around the recurrence.

Sharding: data-parallel over batch. 64 samples -> 8 cores x 8 samples.

Key idea vs the per-step baseline: the memory update is elementwise
per (b, m, d):  x_t = g_t * x_{t-1} + u_t   with g = 1 - w*e, u = w*a.
That maps to the DVE tensor_tensor_scan instruction (state = d0*state
+ d1 along the free axis), so the whole T-loop becomes a handful of
full-tensor passes instead of 200 small dependent instructions:

  layout: partition = (s2, d64), free = (m, t)  [8 samples = s2 x g4]
  PE  : Wbc broadcast of w over d-partitions (indicator matmul, fp16)
  ACT : PSUM->SBUF fp16 copy of Wbc;  g = 1 + (-we)  (Copy bias trick)
  Pool: -we = Wb*E ;  u = Wb*A        (stt with stride-0 e/a broadcast)
  DVE : scan (g,u)->x ; q = x*Wb ; tree-reduce q over m -> reads
  per-recurrence reset slots (g=0, u=Mv0) let one scan instruction
  chain all 32 recurrences of a chunk.

Rows for the parallel head/tail phases are SAMPLE-MAJOR with
per-sample pad: row = b_loc*208 + t (b_loc = g*2 + s; t in [0,208)).
"""

import numpy as np

import concourse.bass as bass
import concourse.bacc as bacc
import concourse.tile as tile
from concourse import mybir
from concourse.bass_utils import run_bass_kernel_spmd

F32 = mybir.dt.float32
F16 = mybir.dt.float16
I32 = mybir.dt.int32
AL = mybir.AluOpType
AF = mybir.ActivationFunctionType
AX = mybir.AxisListType

NUM_SKILLS = 1000
D = 64          # dim_s
M = 64          # size_m
B = 64          # global batch
T = 200
TP = 208        # padded per-sample length
TS = T + 1      # scan slots per recurrence (slot 0 = reset)
NCORES = 8
BL = B // NCORES          # samples per core = 8
RPAD = BL * TP            # 1664
NBLK = RPAD // 128        # 13
RC = 32                   # recurrences (m values) per stage-B chunk
NH = M // RC              # m-halves per g-group = 2


def _build():
    nc = bacc.Bacc(None, target_bir_lowering=False, debug=False)

    # ---- external I/O ----
    d_idxk = nc.dram_tensor("idxk", [RPAD], I32, kind="ExternalInput")
    d_resp = nc.dram_tensor("resp", [RPAD], I32, kind="ExternalInput")
    d_kemb = nc.dram_tensor("kemb", [NUM_SKILLS, D], F32, kind="ExternalInput")
    d_vemb = nc.dram_tensor("vemb", [2 * NUM_SKILLS, D], F32, kind="ExternalInput")
    d_mkt = nc.dram_tensor("mkt", [D, M], F32, kind="ExternalInput")      # Mk^T
    d_eawt = nc.dram_tensor("eawt", [D, 2 * D], F32, kind="ExternalInput")  # [eW^T|aW^T]
    d_fwt = nc.dram_tensor("fwt", [2 * D, D], F32, kind="ExternalInput")  # fW^T
    d_pwb = nc.dram_tensor("pwb", [128, D], F32, kind="ExternalInput")    # pW bcast
    d_ebc = nc.dram_tensor("ebc", [D, 1], F32, kind="ExternalInput")      # eb col
    d_abc = nc.dram_tensor("abc", [D, 1], F32, kind="ExternalInput")      # ab col
    d_fbb = nc.dram_tensor("fbb", [128, D], F32, kind="ExternalInput")    # fb bcast
    d_pbc = nc.dram_tensor("pbc", [128, 1], F32, kind="ExternalInput")    # pb col
    d_ind2 = nc.dram_tensor("ind2", [2, 128], F16, kind="ExternalInput")
    d_ident = nc.dram_tensor("ident", [128, 128], F32, kind="ExternalInput")
    d_s016 = nc.dram_tensor("s016", [128, M], F16, kind="ExternalInput")  # Mv0^T rep
    d_out = nc.dram_tensor("out", [BL, T - 1], F32, kind="ExternalOutput")

    # ---- internal DRAM staging ----
    d_w16 = nc.dram_tensor("w16_stg", [RPAD, M], F16)
    d_p = nc.dram_tensor("p_stg", [RPAD], F32)

    with tile.TileContext(nc) as tc:
        import contextlib
        with contextlib.ExitStack() as ctx:
            singles = ctx.enter_context(tc.tile_pool(name="singles", bufs=1))

            t_idxk = singles.tile([128, NBLK], I32)
            t_idxv = singles.tile([128, NBLK], I32)
            t_resp = singles.tile([128, NBLK], I32)
            t_mkt = singles.tile([D, M], F32)
            t_eawt = singles.tile([D, 2 * D], F32)
            t_fwt1 = singles.tile([D, D], F32)
            t_fwt2 = singles.tile([D, D], F32)
            t_pwb = singles.tile([128, D], F32)
            t_ebc = singles.tile([D, 1], F32)
            t_abc = singles.tile([D, 1], F32)
            t_fbb = singles.tile([128, D], F32)
            t_pbc = singles.tile([128, 1], F32)
            t_ind2 = singles.tile([2, 128], F16)
            t_ident = singles.tile([128, 128], F32)
            t_kT = singles.tile([D, RPAD], F32)      # k^T, sample-major cols
            t_e16T = singles.tile([D, RPAD], F16)    # sigmoid(v@eW^T+eb)^T
            t_a16T = singles.tile([D, RPAD], F16)    # tanh(v@aW^T+ab)^T
            t_rdT = singles.tile([D, RPAD], F32)     # reads^T for stage C
            t_E16 = singles.tile([128, 4, T], F16)   # e in [(s,d), g, t]
            t_A16 = singles.tile([128, 4, T], F16)
            t_s016 = singles.tile([128, M], F16)     # Mv0T[(s,d), m]
            t_reads = singles.tile([128, 4, T], F32)
            t_psig = singles.tile([128, NBLK], F32)

            nc.sync.dma_start(out=t_idxk[:], in_=d_idxk[:].rearrange("(c p) -> p c", p=128))
            nc.sync.dma_start(out=t_resp[:], in_=d_resp[:].rearrange("(c p) -> p c", p=128))
            nc.sync.dma_start(out=t_mkt[:], in_=d_mkt[:])
            nc.sync.dma_start(out=t_eawt[:], in_=d_eawt[:])
            nc.sync.dma_start(out=t_fwt1[:], in_=d_fwt[0:D, :])
            nc.sync.dma_start(out=t_fwt2[:], in_=d_fwt[D:2 * D, :])
            nc.sync.dma_start(out=t_pwb[:], in_=d_pwb[:])
            nc.sync.dma_start(out=t_ebc[:], in_=d_ebc[:])
            nc.sync.dma_start(out=t_abc[:], in_=d_abc[:])
            nc.sync.dma_start(out=t_fbb[:], in_=d_fbb[:])
            nc.sync.dma_start(out=t_pbc[:], in_=d_pbc[:])
            nc.sync.dma_start(out=t_ind2[:], in_=d_ind2[:])
            nc.sync.dma_start(out=t_ident[:], in_=d_ident[:])
            nc.sync.dma_start(out=t_s016[:], in_=d_s016[:])

            # v-table index: x = skills + NUM_SKILLS * responses
            nc.vector.tensor_scalar(out=t_idxv[:], in0=t_resp[:], scalar1=NUM_SKILLS,
                                    scalar2=None, op0=AL.mult)
            nc.vector.tensor_tensor(out=t_idxv[:], in0=t_idxv[:], in1=t_idxk[:], op=AL.add)

            # ================= stage A: gathers, w / e^T / a^T =================
            with tc.tile_pool(name="sa_sb", bufs=3) as sa, \
                 tc.tile_pool(name="sa_ps", bufs=2, space="PSUM") as sap:
                for c in range(NBLK):
                    kg = sa.tile([128, D], F32, tag="kg")
                    vg = sa.tile([128, D], F32, tag="vg")
                    nc.gpsimd.indirect_dma_start(
                        out=kg[:], out_offset=None, in_=d_kemb[:],
                        in_offset=bass.IndirectOffsetOnAxis(ap=t_idxk[:, c:c + 1], axis=0))
                    nc.gpsimd.indirect_dma_start(
                        out=vg[:], out_offset=None, in_=d_vemb[:],
                        in_offset=bass.IndirectOffsetOnAxis(ap=t_idxv[:, c:c + 1], axis=0))
                    kTp = sap.tile([D, 128], F32, tag="ktp", space="PSUM")
                    vTp = sap.tile([D, 128], F32, tag="vtp", space="PSUM")
                    nc.tensor.transpose(out=kTp[:], in_=kg[:], identity=t_ident[:])
                    nc.tensor.transpose(out=vTp[:], in_=vg[:], identity=t_ident[:])
                    nc.scalar.copy(out=t_kT[:, c * 128:(c + 1) * 128], in_=kTp[:])
                    vT = sa.tile([D, 128], F32, tag="vt")
                    nc.scalar.copy(out=vT[:], in_=vTp[:])

                    # logits = k @ Mk^T  -> [128 rows, M]
                    lg = sap.tile([128, M], F32, tag="lg", space="PSUM")
                    nc.tensor.matmul(lg[:], lhsT=t_kT[:, c * 128:(c + 1) * 128],
                                     rhs=t_mkt[:], start=True, stop=True)
                    mx = sa.tile([128, 1], F32, tag="mx")
                    nc.vector.tensor_reduce(out=mx[:], in_=lg[:], axis=AX.X,
                                            op=AL.max, negate=True)
                    wexp = sa.tile([128, M], F32, tag="wexp")
                    sme = sa.tile([128, 1], F32, tag="sme")
                    nc.scalar.activation(out=wexp[:], in_=lg[:], func=AF.Exp,
                                         bias=mx[:], scale=1.0, accum_out=sme[:])
                    rin = sa.tile([128, 1], F32, tag="rin")
                    nc.vector.reciprocal(out=rin[:], in_=sme[:])
                    wb16 = sa.tile([128, M], F16, tag="wb16")
                    nc.vector.tensor_scalar(out=wb16[:], in0=wexp[:], scalar1=rin[:],
                                            scalar2=None, op0=AL.mult)
                    nc.sync.dma_start(out=d_w16[c * 128:(c + 1) * 128, :], in_=wb16[:])

                    # e/a transposed: eaT = [eW^T|aW^T]^T @ v^T -> [(e|a), rows]
                    eaT = sap.tile([2 * D, 128], F32, tag="eat", space="PSUM")
                    nc.tensor.matmul(eaT[:], lhsT=t_eawt[:], rhs=vT[:],
                                     start=True, stop=True)
                    nc.scalar.activation(out=t_e16T[:, c * 128:(c + 1) * 128],
                                         in_=eaT[0:D, :], func=AF.Sigmoid,
                                         bias=t_ebc[:], scale=1.0)
                    nc.scalar.activation(out=t_a16T[:, c * 128:(c + 1) * 128],
                                         in_=eaT[D:2 * D, :], func=AF.Tanh,
                                         bias=t_abc[:], scale=1.0)

            # ============ stage A2: (s,d)-packed e/a tiles ============
            for g in range(4):
                for s in range(2):
                    col = (g * 2 + s) * TP
                    nc.sync.dma_start(out=t_E16[s * D:(s + 1) * D, g, 0:T],
                                      in_=t_e16T[:, col:col + T])
                    nc.sync.dma_start(out=t_A16[s * D:(s + 1) * D, g, 0:T],
                                      in_=t_a16T[:, col:col + T])

            # ================= stage B: scan-based recurrence =================
            # chunk = (g, h): samples pair g, m-half h (RC m-values)
            d_wv = d_w16[:].rearrange("(b t) m -> b t m", b=BL)
            with tc.tile_pool(name="sb_w", bufs=2) as sbw, \
                 tc.tile_pool(name="sb_wb", bufs=2) as sbwb, \
                 tc.tile_pool(name="sb_t", bufs=2) as sbt, \
                 tc.tile_pool(name="sb_s", bufs=2) as sbs, \
                 tc.tile_pool(name="sb_ps", bufs=2, space="PSUM") as sbp:
                for g in range(4):
                    for h in range(NH):
                        m0 = h * RC
                        # -- w chunk load (t-major so the DMA is contiguous;
                        #    split into 4 sub-DMAs for queue parallelism)
                        wT = sbw.tile([2, T, RC], F16, tag="wT")
                        for j in range(4):
                            nc.sync.dma_start(
                                out=wT[:, j * 50:(j + 1) * 50, :],
                                in_=d_wv[g * 2:g * 2 + 2, j * 50:(j + 1) * 50,
                                         m0:m0 + RC])
                        # -- PE: broadcast w over the 64 d-partitions
                        wb = sbwb.tile([128, RC, T], F16, tag="wb")
                        for j in range(4):
                            ps = sbp.tile([128, 4, 512], F32, tag="ps", space="PSUM")
                            for k in range(4):
                                r = j * 4 + k  # m-pair index within chunk
                                nc.tensor.matmul(
                                    ps[:, k, 0:2 * T],
                                    lhsT=t_ind2[:],
                                    rhs=wT[:, :, r * 2:r * 2 + 2].rearrange(
                                        "s t a -> s a t"),
                                    start=True, stop=True)
                            # ACT: PSUM fp32 -> SBUF fp16
                            nc.scalar.activation(
                                out=wb[:, j * 8:(j + 1) * 8, :].rearrange(
                                    "p (a b) t -> p a b t", b=2),
                                in_=ps[:, :, 0:2 * T].rearrange(
                                    "p a (b t) -> p a b t", b=2),
                                func=AF.Copy)

                        # -- Pool: -we ; ACT: g = 1 + (-we) ; Pool: u = w*a
                        ebr = t_E16[:, g, :].unsqueeze(1).broadcast_to([128, RC, T])
                        abr = t_A16[:, g, :].unsqueeze(1).broadcast_to([128, RC, T])
                        tmp = sbt.tile([128, RC, T], F16, tag="tmp")
                        nc.gpsimd.tensor_tensor(out=tmp[:], in0=wb[:], in1=ebr,
                                                op=AL.mult)
                        gt = sbs.tile([128, RC, TS], F16, tag="gt")
                        nc.vector.memset(gt[:, :, 0:1], 0.0)
                        nc.scalar.activation(out=gt[:, :, 1:TS], in_=tmp[:],
                                             func=AF.Copy, bias=1.0, scale=-1.0)
                        ut = sbs.tile([128, RC, TS], F16, tag="ut")
                        nc.scalar.activation(
                            out=ut[:, :, 0:1],
                            in_=t_s016[:, m0:m0 + RC].unsqueeze(2), func=AF.Copy)
                        nc.gpsimd.tensor_tensor(out=ut[:, :, 1:TS], in0=wb[:],
                                                in1=abr, op=AL.mult)

                        # -- DVE: scan -> x (slot j = state after j updates)
                        xt = sbs.tile([128, RC, TS], F16, tag="xt")
                        nc.vector.tensor_tensor_scan(
                            out=xt[:].rearrange("p a b -> p (a b)"),
                            data0=gt[:].rearrange("p a b -> p (a b)"),
                            data1=ut[:].rearrange("p a b -> p (a b)"),
                            initial=0.0, op0=AL.mult, op1=AL.add)

                        # -- DVE: q = x_{t} * w_t ; tree-reduce over m
                        q = sbt.tile([128, RC, T], F16, tag="q")
                        nc.vector.tensor_tensor(out=q[:], in0=xt[:, :, 0:T],
                                                in1=wb[:], op=AL.mult)
                        n = RC
                        while n > 2:
                            n //= 2
                            nc.vector.tensor_tensor(
                                out=q[:, 0:n, :], in0=q[:, 0:n, :],
                                in1=q[:, n:2 * n, :], op=AL.add)
                        if h == 0:
                            nc.vector.tensor_tensor(
                                out=t_reads[:, g, :], in0=q[:, 0, :],
                                in1=q[:, 1, :], op=AL.add)
                        else:
                            s2 = sbt.tile([128, T], F16, tag="s2")
                            nc.vector.tensor_tensor(out=s2[:], in0=q[:, 0, :],
                                                    in1=q[:, 1, :], op=AL.add)
                            nc.vector.tensor_tensor(
                                out=t_reads[:, g, :], in0=t_reads[:, g, :],
                                in1=s2[:], op=AL.add)

            # reads -> [D, RPAD] sample-major for stage C
            for g in range(4):
                for s in range(2):
                    col = (g * 2 + s) * TP
                    nc.sync.dma_start(out=t_rdT[:, col:col + T],
                                      in_=t_reads[s * D:(s + 1) * D, g, 0:T])

            # ================= stage C: output head =================
            with tc.tile_pool(name="sc_sb", bufs=3) as sc, \
                 tc.tile_pool(name="sc_ps", bufs=2, space="PSUM") as scp:
                for c in range(NBLK):
                    fp = scp.tile([128, D], F32, tag="fp", space="PSUM")
                    nc.tensor.matmul(fp[:], lhsT=t_rdT[:, c * 128:(c + 1) * 128],
                                     rhs=t_fwt1[:], start=True, stop=False)
                    nc.tensor.matmul(fp[:], lhsT=t_kT[:, c * 128:(c + 1) * 128],
                                     rhs=t_fwt2[:], start=False, stop=True)
                    fb = sc.tile([128, D], F32, tag="fb")
                    nc.vector.tensor_tensor(out=fb[:], in0=fp[:], in1=t_fbb[:], op=AL.add)
                    ft = sc.tile([128, D], F32, tag="ft")
                    nc.scalar.activation(out=ft[:], in_=fb[:], func=AF.Tanh)
                    junk = sc.tile([128, D], F32, tag="junk")
                    nc.vector.scalar_tensor_tensor(
                        out=junk[:], in0=ft[:], scalar=1.0, in1=t_pwb[:],
                        op0=AL.mult, op1=AL.mult,
                        accum_out=t_psig[:, c:c + 1])
                nc.scalar.activation(out=t_psig[:], in_=t_psig[:], func=AF.Sigmoid,
                                     bias=t_pbc[:], scale=1.0)
                nc.sync.dma_start(out=d_p[:].rearrange("(c p) -> p c", p=128), in_=t_psig[:])

                # out[b, j] = p[b*208 + 1 + j]
                ob = sc.tile([BL, T - 1], F32, tag="ob")
                nc.sync.dma_start(
                    out=ob[:],
                    in_=d_p[:].rearrange("(b t) -> b t", b=BL)[:, 1:T])
                nc.sync.dma_start(out=d_out[:], in_=ob[:])

    nc.compile()
    return nc


_NC_CACHE = None


def _get_nc():
    global _NC_CACHE
    if _NC_CACHE is None:
        _NC_CACHE = _build()
    return _NC_CACHE


def kernel(skills, responses, k_emb, v_emb, Mk, Mv0, fW, fb, eW, eb, aW, ab, pW, pb):
    skills = np.asarray(skills)
    responses = np.asarray(responses)
    k_emb = np.asarray(k_emb, dtype=np.float32)
    v_emb = np.asarray(v_emb, dtype=np.float32)
    Mk = np.asarray(Mk, dtype=np.float32)
    Mv0 = np.asarray(Mv0, dtype=np.float32)
    fW = np.asarray(fW, dtype=np.float32)
    fb = np.asarray(fb, dtype=np.float32)
    eW = np.asarray(eW, dtype=np.float32)
    eb = np.asarray(eb, dtype=np.float32)
    aW = np.asarray(aW, dtype=np.float32)
    ab = np.asarray(ab, dtype=np.float32)
    pW = np.asarray(pW, dtype=np.float32)
    pb = np.asarray(pb, dtype=np.float32)

    mkt = np.ascontiguousarray(Mk.T)                                   # [D, M]
    eawt = np.ascontiguousarray(np.concatenate([eW.T, aW.T], axis=1))  # [D, 2D]
    fwt = np.ascontiguousarray(fW.T)                                   # [2D, D]
    pwb = np.broadcast_to(pW, (128, D)).copy()
    ebc = np.ascontiguousarray(eb.reshape(D, 1))
    abc = np.ascontiguousarray(ab.reshape(D, 1))
    fbb = np.broadcast_to(fb[None, :], (128, D)).copy()
    pbc = np.full((128, 1), float(pb[0]), np.float32)
    ind2 = np.zeros((2, 128), np.float16)
    ind2[0, :64] = 1.0
    ind2[1, 64:] = 1.0
    ident = np.eye(128, dtype=np.float32)
    # s016[(s,d), m] = Mv0[m, d]
    s016 = np.concatenate([Mv0.T, Mv0.T], axis=0).astype(np.float16)

    shared = dict(kemb=k_emb, vemb=v_emb, mkt=mkt, eawt=eawt, fwt=fwt,
                  pwb=pwb, ebc=ebc, abc=abc, fbb=fbb, pbc=pbc, ind2=ind2,
                  ident=ident, s016=s016)

    in_maps = []
    for core in range(NCORES):
        sk = skills[core * BL:(core + 1) * BL].astype(np.int32)
        rs = responses[core * BL:(core + 1) * BL].astype(np.int32)
        idxk = np.zeros((BL, TP), np.int32)
        resp = np.zeros((BL, TP), np.int32)
        idxk[:, :T] = sk          # row = b*208 + t
        resp[:, :T] = rs
        m = dict(shared)
        m["idxk"] = idxk.reshape(-1)
        m["resp"] = resp.reshape(-1)
        in_maps.append(m)

    nc = _get_nc()
    res = run_bass_kernel_spmd(nc, in_maps, core_ids=list(range(NCORES)),
                               **_RUN_KWARGS)
    out = np.concatenate([res.results[i]["out"] for i in range(NCORES)], axis=0)
    global _LAST_RESULT
    _LAST_RESULT = res
    return out.astype(np.float32)


_RUN_KWARGS = {}
_LAST_RESULT = None


def run_traced(**inputs):
    """Run once with NTFF tracing; returns exec_time_ns (or None)."""
    global _RUN_KWARGS
    _RUN_KWARGS = {"trace": True}
    try:
        kernel(**inputs)
    finally:
        _RUN_KWARGS = {}
    return _LAST_RESULT.exec_time_ns if _LAST_RESULT is not None else None
